# revision 1
# baseline (speedup 1.0000x reference)
"""CTC loss kernel for Trainium2, 8-way data parallel over the batch.

Algorithm (per core, 32 examples): the CTC forward DP is computed s-major —
for each extended-label state s (193 of them), the full time recursion over a
T-segment runs as one `tensor_tensor_scan` (state = (d0 + state) * d1) on the
vector engine, batched over the 32 examples on partitions.  The DP dependency
is triangular in s, so a single ascending-s sweep per T-segment is exact.

Emissions are blank-normalized (E = exp(x[label] - x[blank] + ln_kappa)); the
log-softmax denominator cancels and only a bulk sum of per-(b,t) logsumexp is
needed for the final loss, computed via ACT exp + PE ones-matmul + ACT ln.
Gathered label logits come from a Δ-one-hot matmul on the tensor engine.
T is processed in 4 segments with a per-example max renorm between segments
to keep fp32 in range.
"""
import numpy as np
from contextlib import ExitStack

B, T, C, L = 256, 512, 128, 96
S = 2 * L + 1
NCORES = 8
BC = B // NCORES          # 32 examples per core
NSEG = 4
TSEG = T // NSEG          # 128
W = TSEG + 1              # row window incl. boundary col
LN_KAPPA = -1.25
LN2_32 = 32 * float(np.log(2.0))

_cache = {}


def _build():
    import concourse.bass as bass
    import concourse.bacc as bacc
    import concourse.tile as tile
    import concourse.mybir as mybir

    f32 = mybir.dt.float32
    bf16 = mybir.dt.bfloat16
    add = mybir.AluOpType.add
    mult = mybir.AluOpType.mult
    AF = mybir.ActivationFunctionType

    nc = bacc.Bacc("TRN2", target_bir_lowering=False, debug=False,
                   num_devices=NCORES)

    xT_d = nc.dram_tensor("xT", [BC, C, T], f32, kind="ExternalInput")
    oh_d = nc.dram_tensor("oh", [BC, C, L], f32, kind="ExternalInput")
    skm_d = nc.dram_tensor("skm", [BC, L], f32, kind="ExternalInput")
    sel_d = nc.dram_tensor("sel", [BC * NSEG, BC], f32, kind="ExternalInput")
    out_d = nc.dram_tensor("dev_out", [BC, 1], f32, kind="ExternalOutput")
    dbg_d = nc.dram_tensor("dbg", [BC, 8], f32, kind="ExternalOutput")
    dbge_d = nc.dram_tensor("dbg_endc", [BC, S], f32, kind="ExternalOutput")
    dbgl_d = nc.dram_tensor("dbg_lnall", [TSEG, BC * NSEG], f32, kind="ExternalOutput")

    with tile.TileContext(nc, num_cores=NCORES) as tc, ExitStack() as ctx:
        persist = ctx.enter_context(tc.tile_pool(name="persist", bufs=1))
        epool = ctx.enter_context(tc.tile_pool(name="ebuf", bufs=2))
        xtpool = ctx.enter_context(tc.tile_pool(name="xt", bufs=3))
        espool = ctx.enter_context(tc.tile_pool(name="es", bufs=3))
        expool = ctx.enter_context(tc.tile_pool(name="explse", bufs=3))
        upool = ctx.enter_context(tc.tile_pool(name="ustt", bufs=3))
        psumG = ctx.enter_context(
            tc.tile_pool(name="psumG", bufs=2, space=bass.MemorySpace.PSUM))
        psumL = ctx.enter_context(
            tc.tile_pool(name="psumL", bufs=1, space=bass.MemorySpace.PSUM))

        # persistent tiles
        R = persist.tile([BC, S * W], f32)        # alpha segment buffer
        Rv = R[:].rearrange("p (s w) -> p s w", w=W)
        Z = persist.tile([BC, W], f32)            # zero guard row
        ONES = persist.tile([BC, 1], f32)
        KT = persist.tile([BC, TSEG], f32)        # kappa multiplier stream
        SKM = persist.tile([BC, L], f32)
        LOGS = persist.tile([BC, 1], f32)         # accumulated log scales
        SUMLSE = persist.tile([BC, 1], f32)
        OHALL = persist.tile([C, BC * L], f32)    # all Δ-one-hots
        ONESC = persist.tile([C, 1], f32)         # ones rhs for lse matmuls
        KBIAS = persist.tile([L, 1], f32)         # ln_kappa bias for Exp
        SEL = persist.tile([BC * NSEG, BC], f32)  # (e,q)->e selector lhsT
        SEALL = psumL.tile([TSEG, BC * NSEG], f32)  # sum_c exp, col = e*NSEG+q

        nc.vector.memset(Z[:], 0.0)
        nc.vector.memset(ONES[:], 1.0)
        nc.vector.memset(KT[:], float(np.exp(np.float32(LN_KAPPA))))
        nc.vector.memset(LOGS[:], 0.0)
        nc.vector.memset(ONESC[:], 1.0)
        nc.vector.memset(KBIAS[:], float(LN_KAPPA))
        nc.vector.memset(Rv[:, :, 0], 0.0)        # segment-0 boundary cols
        nc.sync.dma_start(SKM[:], skm_d[:])
        nc.sync.dma_start(SEL[:], sel_d[:])
        for e in range(BC):
            nc.sync.dma_start(OHALL[:, e * L:(e + 1) * L], oh_d[e][:])

        def row_win(s):
            """[BC, TSEG] window of row s shifted one t left (cols 0..TSEG-1)."""
            if s < 0:
                return Z[:, 0:TSEG]
            return Rv[:, s, 0:TSEG]

        i32 = mybir.dt.int32
        LN2 = float(np.log(2.0))

        def safe_ln(out, src):
            """out = ln(src) for positive fp32 spanning ~[1e-38, 1e38].
            ACT Ln clamps outside ~[2^-64, 2^64]; two Sqrts compress the
            range into the valid domain: ln(x) = 4*ln(x^(1/4))."""
            S1 = persist.tile([BC, 1], f32, tag="sl_s1")
            nc.scalar.activation(S1[:], src, AF.Sqrt)
            S2 = persist.tile([BC, 1], f32, tag="sl_s2")
            nc.scalar.activation(S2[:], S1[:], AF.Sqrt)
            LnQ = persist.tile([BC, 1], f32, tag="sl_lnq")
            nc.scalar.activation(LnQ[:], S2[:], AF.Ln)
            nc.vector.tensor_scalar_mul(out, LnQ[:], 4.0)

        for q in range(NSEG):
            t0 = q * TSEG
            # ---- produce E rows for this segment (PE/ACT/DMA) ----
            Ebuf = epool.tile([BC, L * TSEG], bf16, tag="ebuf")
            ESALL = espool.tile([L, BC * TSEG], bf16, tag="es")
            for e in range(BC):
                xt = xtpool.tile([C, TSEG], f32, tag="xt")
                nc.sync.dma_start(xt[:], xT_d[e][:, t0:t0 + TSEG])
                G = psumG.tile([L, TSEG], f32, tag="G")
                nc.tensor.matmul(G[:], OHALL[:, e * L:(e + 1) * L], xt[:],
                                 start=True, stop=True)
                nc.scalar.activation(ESALL[:, e * TSEG:(e + 1) * TSEG], G[:],
                                     AF.Exp, bias=KBIAS[:])
                # lse contribution: sum_c exp(x[:, t]) -> SEALL[:, e*NSEG+q]
                EX = expool.tile([C, TSEG], f32, tag="ex")
                nc.scalar.activation(EX[:], xt[:], AF.Exp)
                col = e * NSEG + q
                nc.tensor.matmul(SEALL[:, col:col + 1], EX[:], ONESC[:],
                                 start=True, stop=True,
                                 skip_group_check=True)
            # cross-partition reshuffle DMAs: [l, t] slice -> one Ebuf row
            for e in range(BC):
                nc.sync.dma_start(Ebuf[e:e + 1, :],
                                  ESALL[:, e * TSEG:(e + 1) * TSEG])

            # ---- segment boundary: renorm + boundary cols (DVE/ACT) ----
            if q > 0:
                m = persist.tile([BC, 1], f32, tag="m")
                nc.vector.tensor_reduce(m[:], Rv[:, :, TSEG],
                                        mybir.AxisListType.X,
                                        mybir.AluOpType.max)
                r = persist.tile([BC, 1], f32, tag="r")
                nc.vector.reciprocal(r[:], m[:])
                nc.vector.tensor_scalar_mul(Rv[:, :, 0], Rv[:, :, TSEG], r[:])
                lm = persist.tile([BC, 1], f32, tag="lm")
                safe_ln(lm[:], m[:])
                nc.vector.tensor_tensor(LOGS[:], LOGS[:], lm[:], add)

            # Absorb the Ebuf-DMA dependency into a TT op (the scan/STT ISA
            # struct has a single sync-wait slot, fully used by the DVE
            # self-ordering sem).  bypass keeps SKM's value; the first STT
            # reads SKM[:, 0:1], forcing this op ahead of the s-sweep.
            nc.vector.tensor_tensor(SKM[:, 0:1], SKM[:, 0:1], Ebuf[:, 0:1],
                                    mybir.AluOpType.bypass)

            # ---- the s-sweep: one scan per row (DVE) ----
            for s in range(S):
                if q == 0 and s < 2:
                    init = ONES[:, 0:1]
                else:
                    init = Rv[:, s, 0:1]
                dst = Rv[:, s, 1:W]
                if s % 2 == 0:
                    nc.vector.tensor_tensor_scan(
                        dst, row_win(s - 1), KT[:], init, add, mult)
                else:
                    l = (s - 1) // 2
                    U = upool.tile([BC, TSEG], f32, tag="u")
                    nc.vector.scalar_tensor_tensor(
                        U[:], row_win(s - 2), SKM[:, l:l + 1], row_win(s - 1),
                        mult, add)
                    nc.vector.tensor_tensor_scan(
                        dst, U[:], Ebuf[:, l * TSEG:(l + 1) * TSEG],
                        init, add, mult)

        # ---- sum_t lse per example: Ln, then two partition-contractions ----
        LNALL = persist.tile([TSEG, BC * NSEG], f32)
        nc.scalar.activation(LNALL[:], SEALL[:], AF.Ln)
        SLQ = psumG.tile([BC * NSEG, 1], f32, tag="slq")
        nc.tensor.matmul(SLQ[:], LNALL[:], ONESC[:], start=True, stop=True)
        SLQS = persist.tile([BC * NSEG, 1], f32)
        nc.vector.tensor_copy(SLQS[:], SLQ[:])
        SUMLP = psumG.tile([BC, 1], f32, tag="sumlp")
        nc.tensor.matmul(SUMLP[:], SEL[:], SLQS[:], start=True, stop=True)
        nc.vector.tensor_copy(SUMLSE[:], SUMLP[:])

        # ---- readout ----
        V = persist.tile([BC, 1], f32, tag="v")
        nc.vector.tensor_tensor(V[:], Rv[:, S - 1, TSEG:TSEG + 1],
                                Rv[:, S - 2, TSEG:TSEG + 1], add)
        logV = persist.tile([BC, 1], f32, tag="logv")
        safe_ln(logV[:], V[:])
        dev = persist.tile([BC, 1], f32, tag="dev")
        nc.vector.tensor_tensor(dev[:], logV[:], LOGS[:], add)
        nc.vector.tensor_tensor(dev[:], dev[:], SUMLSE[:],
                                mybir.AluOpType.subtract)
        nc.sync.dma_start(out_d[:], dev[:])
        DBG = persist.tile([BC, 8], f32)
        nc.vector.tensor_copy(DBG[:, 0:1], SUMLSE[:])
        nc.vector.tensor_copy(DBG[:, 1:2], LOGS[:])
        nc.vector.tensor_copy(DBG[:, 2:3], logV[:])
        nc.vector.tensor_copy(DBG[:, 3:4], V[:])
        nc.vector.tensor_copy(DBG[:, 4:5], SLQS[0:BC, :])
        nc.vector.tensor_copy(DBG[:, 5:6], Rv[:, S - 1, TSEG:TSEG + 1])
        nc.vector.tensor_copy(DBG[:, 6:7], Rv[:, S - 2, TSEG:TSEG + 1])
        nc.vector.tensor_copy(DBG[:, 7:8], ONES[:])
        nc.sync.dma_start(dbg_d[:], DBG[:])
        ENDC = persist.tile([BC, S], f32)
        nc.vector.tensor_copy(ENDC[:], Rv[:, :, TSEG])
        nc.sync.dma_start(dbge_d[:], ENDC[:])
        nc.sync.dma_start(dbgl_d[:], LNALL[:])

    nc.compile()
    return nc


def _host_prep(y_pred, y_true):
    y_pred = np.ascontiguousarray(np.asarray(y_pred), dtype=np.float32)
    lab = np.asarray(y_true).astype(np.int32)
    xT = np.ascontiguousarray(y_pred.transpose(0, 2, 1))          # [B, C, T]
    oh = np.zeros((B, C, L), np.float32)
    oh[:, 0, :] = -1.0
    np.put_along_axis(oh, lab[:, None, :], 1.0, axis=1)
    skm = np.ones((B, L), np.float32)
    skm[:, 1:] = (lab[:, 1:] != lab[:, :-1]).astype(np.float32)
    blanksum = y_pred[:, :, 0].astype(np.float64).sum(axis=1)     # [B]
    sel = (np.arange(BC * NSEG)[:, None] // NSEG
           == np.arange(BC)[None, :]).astype(np.float32)
    return xT, oh, skm, sel, blanksum


def kernel(y_pred, y_true, _trace=False):
    from concourse.bass_utils import run_bass_kernel_spmd

    xT, oh, skm, sel, blanksum = _host_prep(y_pred, y_true)
    if "nc" not in _cache:
        _cache["nc"] = _build()
    nc = _cache["nc"]

    in_maps = []
    for i in range(NCORES):
        sl = slice(i * BC, (i + 1) * BC)
        in_maps.append({"xT": xT[sl], "oh": oh[sl], "skm": skm[sl],
                        "sel": sel})
    res = run_bass_kernel_spmd(nc, in_maps, core_ids=list(range(NCORES)),
                               trace=_trace)
    _cache["last_result"] = res
    dev = np.concatenate([r["dev_out"][:, 0] for r in res.results])   # [B]
    loss = -(dev.astype(np.float64) - T * LN_KAPPA + blanksum)
    return loss.astype(np.float32)

